# revision 8
# baseline (speedup 1.0000x reference)
"""MultiHeadAttention Trainium2 kernel (8 NeuronCores, SPMD).

Sharding: core c -> (batch b = c//2, head-group hg = c%2 of 8 heads).
Each core computes q/k/v projections for its 512 head-columns, attention
for its 8 heads, and a partial output projection over its 512 rows of Wo.
Host sums the two partials per batch and adds bo.

Mask insight: the mask is a padding mask over key positions (mask=1 adds
-1e9 to the whole logit column -> exp underflows to exactly 0 in fp32,
identically to the reference).  Masked K/V rows therefore contribute
nothing; the host gathers only unmasked rows, padding to a multiple of
256 with rows whose logits are forced to -1e9 via a per-partition bias.
This halves attention FLOPs and exp work on average.

All matmuls run in float32r (fp32 stored/streamed in a PE-friendly form,
full bf16-rate at moving-dim>=256, ~1.5e-4 matmul error vs 2.4e-3 bf16).
"""

import numpy as np

import concourse.bass as bass  # noqa: F401  (bass registers engine types)
import concourse.mybir as mybir
from concourse import bacc
from concourse.tile import TileContext
from concourse.bass_utils import run_bass_kernel_spmd

D = 1024
H = 16
DH = 64
B = 4
S = 2048
NCORE = 8
HPC = 8            # heads per core
COLS = HPC * DH    # 512 projection columns per core
SCALE = 1.0 / np.sqrt(DH)

f32 = mybir.dt.float32
f32r = mybir.dt.float32r
Exp = mybir.ActivationFunctionType.Exp
Ident = mybir.ActivationFunctionType.Identity

_prog_cache = {}


def _k_chunks(n):
    """Split n (multiple of 256) into chunks <=512, each >=256."""
    return [512] * (n // 512) + ([256] if n % 512 else [])


def _build(skp):
    nkt = skp // 128
    kch = _k_chunks(skp)

    nc = bacc.Bacc("TRN2", target_bir_lowering=False, debug=False,
                   num_devices=NCORE)

    xqT = nc.dram_tensor("xqT", [D, S], f32r, kind="ExternalInput")
    xkT = nc.dram_tensor("xkT", [D, skp], f32r, kind="ExternalInput")
    xvT = nc.dram_tensor("xvT", [D, skp], f32r, kind="ExternalInput")
    wq = nc.dram_tensor("wq", [D, COLS], f32r, kind="ExternalInput")
    wk = nc.dram_tensor("wk", [D, COLS], f32r, kind="ExternalInput")
    wvx = nc.dram_tensor("wvx", [D, HPC * 65], f32r, kind="ExternalInput")
    wo = nc.dram_tensor("wo", [COLS, D], f32r, kind="ExternalInput")
    bq = nc.dram_tensor("bq", [COLS], f32, kind="ExternalInput")
    bk = nc.dram_tensor("bk", [COLS], f32, kind="ExternalInput")
    bvx = nc.dram_tensor("bvx", [HPC * 65], f32r, kind="ExternalInput")
    mbias = nc.dram_tensor("mbias", [skp], f32, kind="ExternalInput")
    onesd = nc.dram_tensor("onesd", [128, 128], f32r, kind="ExternalInput")
    part = nc.dram_tensor("part", [S, D], f32, kind="ExternalOutput")

    with TileContext(nc) as tc, \
         nc.allow_low_precision(reason="float32r matmul pipeline by design"):
        with tc.tile_pool(name="resid", bufs=1) as rp:
            # long-lived SBUF residents
            qT = [rp.tile([128, S], f32r, tag=f"qT{p}", name=f"qT{p}")
                  for p in range(4)]
            kT = [rp.tile([128, skp], f32r, tag=f"kT{p}", name=f"kT{p}")
                  for p in range(4)]
            vE = [[rp.tile([128, 4 * 65], f32r, tag=f"vE{c}_{t}",
                           name=f"vE{c}_{t}")
                   for t in range(nkt)] for c in range(2)]
            OT = [rp.tile([128, S], f32r, tag=f"OT{p}", name=f"OT{p}")
                  for p in range(4)]
            ones = rp.tile([128, 128], f32r, tag="ones")
            nc.sync.dma_start(out=ones[:], in_=onesd[:])
            mbt = []
            for t in range(nkt):
                m = rp.tile([128, 1], f32, tag=f"mb{t}")
                nc.sync.dma_start(out=m[:], in_=mbias[t * 128:(t + 1) * 128][:, None])
                mbt.append(m)
            qbt, kbt = [], []
            for m in range(4):
                tqb = rp.tile([128, 1], f32, tag=f"qb{m}")
                nc.sync.dma_start(out=tqb[:], in_=bq[m * 128:(m + 1) * 128][:, None])
                qbt.append(tqb)
                tkb = rp.tile([128, 1], f32, tag=f"kb{m}")
                nc.sync.dma_start(out=tkb[:], in_=bk[m * 128:(m + 1) * 128][:, None])
                kbt.append(tkb)
            bvt = rp.tile([128, HPC * 65], f32r, tag="bvt")
            nc.sync.dma_start(out=bvt[0:1, :], in_=bvx[None, :])

            # ---- Phase A/B: q^T and k^T projections (weights stationary) --
            for (wdr, xdr, dst, bias_t, chunks) in (
                    (wq, xqT, qT, qbt, [512] * 4),
                    (wk, xkT, kT, kbt, kch)):
                with (tc.tile_pool(name="wst", bufs=34) as wp,
                      tc.tile_pool(name="xmov", bufs=3) as xp,
                      tc.tile_pool(name="psA", bufs=2, space="PSUM") as pp):
                    wt = [[None] * 8 for _ in range(4)]
                    for m in range(4):
                        for k in range(8):
                            w = wp.tile([128, 128], f32r, tag="w")
                            nc.sync.dma_start(
                                out=w[:],
                                in_=wdr[k * 128:(k + 1) * 128, m * 128:(m + 1) * 128])
                            wt[m][k] = w
                    off = 0
                    for csz in chunks:
                        xts = []
                        for k in range(8):
                            xt = xp.tile([128, 512], f32r, tag="x")
                            nc.sync.dma_start(out=xt[:, :csz],
                                              in_=xdr[k * 128:(k + 1) * 128,
                                                      off:off + csz])
                            xts.append(xt)
                        for m in range(4):
                            ps = pp.tile([128, 512], f32, tag=f"ps{m}")
                            for k in range(8):
                                nc.tensor.matmul(ps[:, :csz], wt[m][k][:],
                                                 xts[k][:, :csz],
                                                 start=(k == 0), stop=(k == 7))
                            nc.scalar.activation(dst[m][:, off:off + csz],
                                                 ps[:, :csz], Ident,
                                                 bias=bias_t[m][:])
                        off += csz

            # ---- Phase C: v projection, natural layout + ones columns ----
            # wvx columns: per head h: 64 v-columns then one zero column;
            # bvx has 1.0 at the ones slots -> psum = v | sums-ones directly.
            with (tc.tile_pool(name="wvp", bufs=8) as wvp,
                  tc.tile_pool(name="xvst", bufs=3) as xvp,
                  tc.tile_pool(name="psC", bufs=2, space="PSUM") as pp):
                wvt = []
                for k in range(8):
                    w = wvp.tile([128, HPC * 65], f32r, tag="wv")
                    nc.sync.dma_start(out=w[:], in_=wvx[k * 128:(k + 1) * 128, :])
                    wvt.append(w)
                for t in range(nkt):
                    xvs = []
                    for k in range(8):
                        xt = xvp.tile([128, 128], f32r, tag="xv")
                        nc.sync.dma_start(out=xt[:],
                                          in_=xvT[k * 128:(k + 1) * 128,
                                                  t * 128:(t + 1) * 128])
                        xvs.append(xt)
                    for c in range(2):
                        ps = pp.tile([128, 260], f32, tag=f"c{c}")
                        sl = slice(c * 260, (c + 1) * 260)
                        for k in range(8):
                            nc.tensor.matmul(ps[:], xvs[k][:], wvt[k][:, sl],
                                             start=(k == 0), stop=False)
                        # bias + ones row via K=1 matmul with a ones row
                        nc.tensor.matmul(ps[:], ones[0:1, :], bvt[0:1, sl],
                                         start=False, stop=True)
                        nc.scalar.activation(vE[c][t][:], ps[:], Ident)

            # ---- Phase D: attention per head pair ----
            with (tc.tile_pool(name="pT", bufs=4) as ptp,
                  tc.tile_pool(name="nrm", bufs=4) as nrp,
                  tc.tile_pool(name="psL", bufs=2, space="PSUM") as psL,
                  tc.tile_pool(name="psO", bufs=1, space="PSUM") as psO):
                for p in range(4):
                    hA, hB = 2 * p, 2 * p + 1
                    vA = (vE[hA // 4], (hA % 4) * 65)
                    vB = (vE[hB // 4], (hB % 4) * 65)
                    for qc in range(2):
                        q0 = qc * 1024
                        oA = psO.tile([65, 1024], f32, tag="oA")
                        oB = psO.tile([65, 1024], f32, tag="oB")
                        for t in range(nkt):
                            ks = slice(t * 128, (t + 1) * 128)
                            psA = psL.tile([128, 1024], f32, tag="ps")
                            psB = psL.tile([128, 1024], f32, tag="ps")
                            for h2 in range(2):
                                qs = slice(q0 + h2 * 512, q0 + (h2 + 1) * 512)
                                ds = slice(h2 * 512, (h2 + 1) * 512)
                                nc.tensor.matmul(psA[:, ds], kT[p][0:64, ks],
                                                 qT[p][0:64, qs], start=True,
                                                 stop=True, tile_position=(0, 0))
                                nc.tensor.matmul(psB[:, ds], kT[p][64:128, ks],
                                                 qT[p][64:128, qs], start=True,
                                                 stop=True, tile_position=(64, 0))
                            pA = ptp.tile([128, 1024], f32r, tag="pT")
                            pB = ptp.tile([128, 1024], f32r, tag="pT")
                            nc.scalar.activation(pA[:], psA[:], Exp,
                                                 bias=mbt[t][:], scale=SCALE)
                            nc.scalar.activation(pB[:], psB[:], Exp,
                                                 bias=mbt[t][:], scale=SCALE)
                            for h2 in range(2):
                                ds = slice(h2 * 512, (h2 + 1) * 512)
                                nc.tensor.matmul(oA[:, ds],
                                                 vA[0][t][:, vA[1]:vA[1] + 65],
                                                 pA[:, ds], start=(t == 0),
                                                 stop=(t == nkt - 1))
                                nc.tensor.matmul(oB[:, ds],
                                                 vB[0][t][:, vB[1]:vB[1] + 65],
                                                 pB[:, ds], start=(t == 0),
                                                 stop=(t == nkt - 1))
                        # normalize: OT rows = o[0:64] * bcast(1/sums)
                        for half, o in ((0, oA), (1, oB)):
                            recr = nrp.tile([128, 1024], f32r, tag="recr")
                            nc.vector.reciprocal(recr[64:65, :], o[64:65, :])
                            bc = psL.tile([64, 1024], f32, tag="ps")
                            for h2 in range(2):
                                ds = slice(h2 * 512, (h2 + 1) * 512)
                                nc.tensor.matmul(bc[:, ds], ones[64:65, 0:64],
                                                 recr[64:65, ds],
                                                 start=True, stop=True)
                            bcs = nrp.tile([64, 1024], f32, tag="bcs")
                            nc.vector.tensor_copy(bcs[:], bc[:])
                            nc.vector.tensor_tensor(
                                out=OT[p][half * 64:(half + 1) * 64,
                                          q0:q0 + 1024],
                                in0=o[0:64, :], in1=bcs[:],
                                op=mybir.AluOpType.mult)

            # ---- Phase E: output projection (partial; host adds pair+bo) --
            with (tc.tile_pool(name="wop", bufs=8) as wop,
                  tc.tile_pool(name="oevac", bufs=3) as oev,
                  tc.tile_pool(name="psE", bufs=3, space="PSUM") as pp):
                wot = [[None] * 2 for _ in range(4)]
                for k in range(4):
                    for n in range(2):
                        w = wop.tile([128, 512], f32r, tag="wo")
                        nc.sync.dma_start(
                            out=w[:], in_=wo[k * 128:(k + 1) * 128,
                                             n * 512:(n + 1) * 512])
                        wot[k][n] = w
                for st in range(16):
                    ss = slice(st * 128, (st + 1) * 128)
                    ps = pp.tile([128, 1024], f32, tag="po")
                    for k in range(4):
                        for n in range(2):
                            nc.tensor.matmul(ps[:, n * 512:(n + 1) * 512],
                                             OT[k][:, ss], wot[k][n][:],
                                             start=(k == 0), stop=(k == 3))
                    ot = oev.tile([128, 1024], f32, tag="oe")
                    nc.vector.tensor_copy(ot[:], ps[:])
                    nc.sync.dma_start(out=part[ss, :], in_=ot[:])

    nc.compile()
    return nc


def prepare(query, key, value, mask, Wq, bq, Wk, bk, Wv, bv, Wo, bo):
    """Build/fetch the compiled program and the per-core input maps."""
    query = np.asarray(query, np.float32)
    key = np.asarray(key, np.float32)
    value = np.asarray(value, np.float32)
    mask = np.asarray(mask, np.float32)
    Wq, bq = np.asarray(Wq, np.float32), np.asarray(bq, np.float32)
    Wk, bk = np.asarray(Wk, np.float32), np.asarray(bk, np.float32)
    Wv, bv = np.asarray(Wv, np.float32), np.asarray(bv, np.float32)
    Wo, bo = np.asarray(Wo, np.float32), np.asarray(bo, np.float32)

    idx = [np.nonzero(mask[b, 0, 0] == 0.0)[0] for b in range(B)]
    sk_max = max(1, max(len(i) for i in idx))
    skp = max(256, int(-(-sk_max // 256) * 256))

    if skp not in _prog_cache:
        _prog_cache[skp] = _build(skp)
    nc = _prog_cache[skp]

    onesv = np.ones((128, 128), np.float32)
    in_maps = []
    for c in range(NCORE):
        b, hg = divmod(c, 2)
        cols = slice(hg * COLS, (hg + 1) * COLS)
        sk = len(idx[b])
        xk = np.zeros((skp, D), np.float32)
        xk[:sk] = key[b][idx[b]]
        xv = np.zeros((skp, D), np.float32)
        xv[:sk] = value[b][idx[b]]
        wv_s = Wv[:, cols]
        bv_s = bv[cols]
        wvx = np.zeros((D, HPC * 65), np.float32)
        bvx = np.zeros(HPC * 65, np.float32)
        for h in range(HPC):
            wvx[:, h * 65:h * 65 + 64] = wv_s[:, h * 64:(h + 1) * 64]
            bvx[h * 65:h * 65 + 64] = bv_s[h * 64:(h + 1) * 64]
            bvx[h * 65 + 64] = 1.0
        mb = np.full(skp, -1e9, np.float32)
        mb[:sk] = 0.0
        in_maps.append({
            "xqT": np.ascontiguousarray(query[b].T),
            "xkT": np.ascontiguousarray(xk.T),
            "xvT": np.ascontiguousarray(xv.T),
            "wq": np.ascontiguousarray(Wq[:, cols]),
            "wk": np.ascontiguousarray(Wk[:, cols]),
            "wvx": wvx,
            "wo": np.ascontiguousarray(Wo[cols, :]),
            "bq": np.ascontiguousarray(bq[cols]),
            "bk": np.ascontiguousarray(bk[cols]),
            "bvx": bvx,
            "mbias": mb,
            "onesd": onesv,
        })

    return nc, in_maps


def kernel(query, key, value, mask, Wq, bq, Wk, bk, Wv, bv, Wo, bo):
    nc, in_maps = prepare(query, key, value, mask, Wq, bq, Wk, bk,
                          Wv, bv, Wo, bo)
    res = run_bass_kernel_spmd(nc, in_maps, list(range(NCORE)))
    bo = np.asarray(bo, np.float32)
    out = np.empty((B, S, D), np.float32)
    for b in range(B):
        out[b] = res.results[2 * b]["part"] + res.results[2 * b + 1]["part"] + bo
    return out


# revision 13
# speedup vs baseline: 1.2430x; 1.2430x over previous
"""MultiHeadAttention Trainium2 kernel (8 NeuronCores, SPMD).

Sharding: core c -> (batch b = c//2, head-group hg = c%2 of 8 heads).
Each core computes q/k/v projections for its 512 head-columns, attention
for its 8 heads, and a partial output projection over its 512 rows of Wo.
Host sums the two partials per batch and adds bo.

Mask insight: the mask is a padding mask over key positions (mask=1 adds
-1e9 to the whole logit column -> exp underflows to exactly 0 in fp32,
identically to the reference).  Masked K/V rows therefore contribute
nothing; the host gathers only unmasked rows, padding to a multiple of
256 with rows whose logits are forced to -1e9 via a per-partition bias.
This halves attention FLOPs and exp work on average.

All matmuls run in float32r (fp32 stored/streamed in a PE-friendly form,
full bf16-rate at moving-dim>=256, ~1.5e-4 matmul error vs 2.4e-3 bf16).
"""

import numpy as np

import concourse.bass as bass  # noqa: F401  (bass registers engine types)
import concourse.mybir as mybir
from concourse import bacc
from concourse.tile import TileContext
from concourse.bass_utils import run_bass_kernel_spmd

D = 1024
H = 16
DH = 64
B = 4
S = 2048
NCORE = 8
HPC = 8            # heads per core
COLS = HPC * DH    # 512 projection columns per core
SCALE = 1.0 / np.sqrt(DH)

f32 = mybir.dt.float32
f32r = mybir.dt.float32r
Exp = mybir.ActivationFunctionType.Exp
Ident = mybir.ActivationFunctionType.Identity

_prog_cache = {}


def _k_chunks(n):
    """Split n (multiple of 256) into chunks <=512, each >=256."""
    return [512] * (n // 512) + ([256] if n % 512 else [])


def _build(skp):
    nkt = skp // 128
    kch = _k_chunks(skp)

    nc = bacc.Bacc("TRN2", target_bir_lowering=False, debug=False,
                   num_devices=NCORE)

    xqT = nc.dram_tensor("xqT", [D, S], f32r, kind="ExternalInput")
    xkT = nc.dram_tensor("xkT", [D, skp], f32r, kind="ExternalInput")
    xvT = nc.dram_tensor("xvT", [D, skp], f32r, kind="ExternalInput")
    wq = nc.dram_tensor("wq", [D, COLS], f32r, kind="ExternalInput")
    wk = nc.dram_tensor("wk", [D, COLS], f32r, kind="ExternalInput")
    wvx = nc.dram_tensor("wvx", [D, HPC * 65], f32r, kind="ExternalInput")
    wo = nc.dram_tensor("wo", [COLS, D], f32r, kind="ExternalInput")
    bqk = nc.dram_tensor("bqk", [128, 8], f32, kind="ExternalInput")
    bvx = nc.dram_tensor("bvx", [HPC * 65], f32r, kind="ExternalInput")
    mbias = nc.dram_tensor("mbias", [128, skp // 128], f32,
                           kind="ExternalInput")
    onesd = nc.dram_tensor("onesd", [128, 128], f32r, kind="ExternalInput")
    part = nc.dram_tensor("part", [S, D], f32, kind="ExternalOutput")

    with TileContext(nc) as tc, \
         nc.allow_low_precision(reason="float32r matmul pipeline by design"):
        with tc.tile_pool(name="resid", bufs=1) as rp:
            # long-lived SBUF residents
            qT = [rp.tile([128, S], f32r, tag=f"qT{p}", name=f"qT{p}")
                  for p in range(4)]
            kT = [rp.tile([128, skp], f32r, tag=f"kT{p}", name=f"kT{p}")
                  for p in range(4)]
            vE = [[rp.tile([128, 4 * 65], f32r, tag=f"vE{c}_{t}",
                           name=f"vE{c}_{t}")
                   for t in range(nkt)] for c in range(2)]
            OT = [rp.tile([128, S], f32r, tag=f"OT{p}", name=f"OT{p}")
                  for p in range(4)]
            ones = rp.tile([128, 128], f32r, tag="ones")
            nc.sync.dma_start(out=ones[:], in_=onesd[:])
            # packed per-partition constants: one DMA each (mbias/bq/bk are
            # shipped host-side as [128, n] column packs)
            mbtile = rp.tile([128, nkt], f32, tag="mbtile")
            nc.sync.dma_start(out=mbtile[:], in_=mbias[:])
            mbt = [mbtile[:, t:t + 1] for t in range(nkt)]
            qkb = rp.tile([128, 8], f32, tag="qkb")
            nc.sync.dma_start(out=qkb[:], in_=bqk[:])
            qbt = [qkb[:, m:m + 1] for m in range(4)]
            kbt = [qkb[:, 4 + m:5 + m] for m in range(4)]
            bvt = rp.tile([128, HPC * 65], f32r, tag="bvt")
            nc.sync.dma_start(out=bvt[0:1, :], in_=bvx[None, :])
            # output-projection weights, resident (prefetch early)
            wot = [[None] * 2 for _ in range(4)]
            for k in range(4):
                for n in range(2):
                    w = rp.tile([128, 512], f32r, tag=f"wo{k}{n}",
                                name=f"wo{k}{n}")
                    nc.sync.dma_start(
                        out=w[:], in_=wo[k * 128:(k + 1) * 128,
                                         n * 512:(n + 1) * 512])
                    wot[k][n] = w

            # ---- Phase A/B: q^T and k^T projections (weights stationary) --
            for (wdr, xdr, dst, bias_t, chunks) in (
                    (wq, xqT, qT, qbt, [512] * 4),
                    (wk, xkT, kT, kbt, kch)):
                with (tc.tile_pool(name="wst", bufs=34) as wp,
                      tc.tile_pool(name="xmov", bufs=3) as xp,
                      tc.tile_pool(name="psA", bufs=2, space="PSUM") as pp):
                    wt = [[None] * 8 for _ in range(4)]
                    for m in range(4):
                        for k in range(8):
                            w = wp.tile([128, 128], f32r, tag="w")
                            nc.sync.dma_start(
                                out=w[:],
                                in_=wdr[k * 128:(k + 1) * 128, m * 128:(m + 1) * 128])
                            wt[m][k] = w
                    off = 0
                    for csz in chunks:
                        xts = []
                        for k in range(8):
                            xt = xp.tile([128, 512], f32r, tag="x")
                            nc.sync.dma_start(out=xt[:, :csz],
                                              in_=xdr[k * 128:(k + 1) * 128,
                                                      off:off + csz])
                            xts.append(xt)
                        for m in range(4):
                            ps = pp.tile([128, 512], f32, tag=f"ps{m}")
                            for k in range(8):
                                nc.tensor.matmul(ps[:, :csz], wt[m][k][:],
                                                 xts[k][:, :csz],
                                                 start=(k == 0), stop=(k == 7))
                            nc.scalar.activation(dst[m][:, off:off + csz],
                                                 ps[:, :csz], Ident,
                                                 bias=bias_t[m])
                        off += csz

            # ---- Phase C: v projection, natural layout + ones columns ----
            # wvx columns: per head h: 64 v-columns then one zero column;
            # bvx has 1.0 at the ones slots -> psum = v | sums-ones directly.
            with (tc.tile_pool(name="wvp", bufs=8) as wvp,
                  tc.tile_pool(name="xvst", bufs=3) as xvp,
                  tc.tile_pool(name="psC", bufs=2, space="PSUM") as pp):
                wvt = []
                for k in range(8):
                    w = wvp.tile([128, HPC * 65], f32r, tag="wv")
                    nc.sync.dma_start(out=w[:], in_=wvx[k * 128:(k + 1) * 128, :])
                    wvt.append(w)
                for t in range(nkt):
                    xvs = []
                    for k in range(8):
                        xt = xvp.tile([128, 128], f32r, tag="xv")
                        nc.sync.dma_start(out=xt[:],
                                          in_=xvT[k * 128:(k + 1) * 128,
                                                  t * 128:(t + 1) * 128])
                        xvs.append(xt)
                    for c in range(2):
                        ps = pp.tile([128, 260], f32, tag=f"c{c}")
                        sl = slice(c * 260, (c + 1) * 260)
                        for k in range(8):
                            nc.tensor.matmul(ps[:], xvs[k][:], wvt[k][:, sl],
                                             start=(k == 0), stop=False)
                        # bias + ones row via K=1 matmul with a ones row
                        nc.tensor.matmul(ps[:], ones[0:1, :], bvt[0:1, sl],
                                         start=False, stop=True)
                        nc.scalar.activation(vE[c][t][:], ps[:], Ident)

            # ---- Phase D+E: attention (pipelined) + interleaved out-proj
            # Per (pair, qc512): logits for heads 2p/2p+1 land in one
            # [128,1024] PSUM tile (row-packed K=64 matmuls), ONE exp
            # covers both heads, PV accumulates [65,512] per head with the
            # ones-column providing softmax denominators.  The t-loop is
            # software-pipelined (logits t+2 emitted between PV steps) so
            # the PE never head-of-line blocks on exp.  After all 4 pairs
            # of a q-chunk, the output projection for those 4 S-tiles runs,
            # giving the PE a dense burst and streaming results out.
            with (tc.tile_pool(name="pT", bufs=3) as ptp,
                  tc.tile_pool(name="nrm", bufs=4) as nrp,
                  tc.tile_pool(name="oevac", bufs=3) as oev,
                  tc.tile_pool(name="psL", bufs=2, space="PSUM") as psL,
                  tc.tile_pool(name="psO", bufs=2, space="PSUM") as psO):
                for qc in range(4):
                    q0 = qc * 512
                    for p in range(4):
                        hA, hB = 2 * p, 2 * p + 1
                        vA = (vE[hA // 4], (hA % 4) * 65)
                        vB = (vE[hB // 4], (hB % 4) * 65)
                        qs = slice(q0, q0 + 512)

                        def lgt(t):
                            ks = slice(t * 128, (t + 1) * 128)
                            L = psL.tile([128, 1024], f32, tag="ps",
                                         name=f"L{t}")
                            nc.tensor.matmul(L[:, 0:512], kT[p][0:64, ks],
                                             qT[p][0:64, qs], start=True,
                                             stop=True, tile_position=(0, 0))
                            nc.tensor.matmul(L[:, 512:1024], kT[p][64:128, ks],
                                             qT[p][64:128, qs], start=True,
                                             stop=True, tile_position=(64, 0))
                            return L

                        oA = psO.tile([65, 512], f32, tag="oA")
                        oB = psO.tile([65, 512], f32, tag="oB")
                        Ls = {0: lgt(0)}
                        if nkt > 1:
                            Ls[1] = lgt(1)
                        for t in range(nkt):
                            pTt = ptp.tile([128, 1024], f32r, tag="pT")
                            nc.scalar.activation(pTt[:], Ls.pop(t)[:], Exp,
                                                 bias=mbt[t], scale=SCALE)
                            nc.tensor.matmul(oA[:],
                                             vA[0][t][:, vA[1]:vA[1] + 65],
                                             pTt[:, 0:512], start=(t == 0),
                                             stop=(t == nkt - 1))
                            nc.tensor.matmul(oB[:],
                                             vB[0][t][:, vB[1]:vB[1] + 65],
                                             pTt[:, 512:1024], start=(t == 0),
                                             stop=(t == nkt - 1))
                            if t + 2 < nkt:
                                Ls[t + 2] = lgt(t + 2)
                        # normalize: OT rows = o[0:64] / sums (via recip(bcast))
                        for half, o in ((0, oA), (1, oB)):
                            sm = nrp.tile([128, 512], f32r, tag="sm")
                            nc.vector.tensor_copy(sm[64:65, :], o[64:65, :])
                            bc = psL.tile([64, 512], f32, tag="ps",
                                          name=f"bc{half}")
                            nc.tensor.matmul(bc[:], ones[64:65, 0:64],
                                             sm[64:65, :], start=True,
                                             stop=True)
                            rec = nrp.tile([64, 512], f32, tag="rec")
                            nc.vector.reciprocal_approx_fast(out=rec[:],
                                                             in_=bc[:])
                            nc.vector.tensor_tensor(
                                out=OT[p][half * 64:(half + 1) * 64, qs],
                                in0=o[0:64, :], in1=rec[:],
                                op=mybir.AluOpType.mult)
                    # out-projection for this q-chunk (4 S-tiles of 128)
                    for st in range(4 * qc, 4 * qc + 4):
                        ss = slice(st * 128, (st + 1) * 128)
                        po = psL.tile([128, 1024], f32, tag="ps",
                                      name=f"po{st}")
                        for k in range(4):
                            for n in range(2):
                                nc.tensor.matmul(po[:, n * 512:(n + 1) * 512],
                                                 OT[k][:, ss], wot[k][n][:],
                                                 start=(k == 0), stop=(k == 3))
                        ot = oev.tile([128, 1024], f32, tag="oe")
                        nc.vector.tensor_copy(ot[:], po[:])
                        nc.sync.dma_start(out=part[ss, :], in_=ot[:])

    nc.compile()
    return nc


def prepare(query, key, value, mask, Wq, bq, Wk, bk, Wv, bv, Wo, bo):
    """Build/fetch the compiled program and the per-core input maps."""
    query = np.asarray(query, np.float32)
    key = np.asarray(key, np.float32)
    value = np.asarray(value, np.float32)
    mask = np.asarray(mask, np.float32)
    Wq, bq = np.asarray(Wq, np.float32), np.asarray(bq, np.float32)
    Wk, bk = np.asarray(Wk, np.float32), np.asarray(bk, np.float32)
    Wv, bv = np.asarray(Wv, np.float32), np.asarray(bv, np.float32)
    Wo, bo = np.asarray(Wo, np.float32), np.asarray(bo, np.float32)

    idx = [np.nonzero(mask[b, 0, 0] == 0.0)[0] for b in range(B)]
    sk_max = max(1, max(len(i) for i in idx))
    skp = max(256, int(-(-sk_max // 256) * 256))

    if skp not in _prog_cache:
        _prog_cache[skp] = _build(skp)
    nc = _prog_cache[skp]

    onesv = np.ones((128, 128), np.float32)
    in_maps = []
    for c in range(NCORE):
        b, hg = divmod(c, 2)
        cols = slice(hg * COLS, (hg + 1) * COLS)
        sk = len(idx[b])
        xk = np.zeros((skp, D), np.float32)
        xk[:sk] = key[b][idx[b]]
        xv = np.zeros((skp, D), np.float32)
        xv[:sk] = value[b][idx[b]]
        wv_s = Wv[:, cols]
        bv_s = bv[cols]
        wvx = np.zeros((D, HPC * 65), np.float32)
        bvx = np.zeros(HPC * 65, np.float32)
        for h in range(HPC):
            wvx[:, h * 65:h * 65 + 64] = wv_s[:, h * 64:(h + 1) * 64]
            bvx[h * 65:h * 65 + 64] = bv_s[h * 64:(h + 1) * 64]
            bvx[h * 65 + 64] = 1.0
        mb = np.full(skp, -1e9, np.float32)
        mb[:sk] = 0.0
        bqk = np.empty((128, 8), np.float32)
        for m in range(4):
            bqk[:, m] = bq[cols][m * 128:(m + 1) * 128]
            bqk[:, 4 + m] = bk[cols][m * 128:(m + 1) * 128]
        in_maps.append({
            "xqT": np.ascontiguousarray(query[b].T),
            "xkT": np.ascontiguousarray(xk.T),
            "xvT": np.ascontiguousarray(xv.T),
            "wq": np.ascontiguousarray(Wq[:, cols]),
            "wk": np.ascontiguousarray(Wk[:, cols]),
            "wvx": wvx,
            "wo": np.ascontiguousarray(Wo[cols, :]),
            "bqk": bqk,
            "bvx": bvx,
            "mbias": np.ascontiguousarray(mb.reshape(-1, 128).T),
            "onesd": onesv,
        })

    return nc, in_maps


def kernel(query, key, value, mask, Wq, bq, Wk, bk, Wv, bv, Wo, bo):
    nc, in_maps = prepare(query, key, value, mask, Wq, bq, Wk, bk,
                          Wv, bv, Wo, bo)
    res = run_bass_kernel_spmd(nc, in_maps, list(range(NCORE)))
    bo = np.asarray(bo, np.float32)
    out = np.empty((B, S, D), np.float32)
    for b in range(B):
        out[b] = res.results[2 * b]["part"] + res.results[2 * b + 1]["part"] + bo
    return out


# revision 17
# speedup vs baseline: 1.5880x; 1.2775x over previous
"""MultiHeadAttention Trainium2 kernel (8 NeuronCores, SPMD).

Sharding: core c -> (batch b = c//2, head-group hg = c%2 of 8 heads).
Each core computes q/k/v projections for its 512 head-columns, attention
for its 8 heads, and a partial output projection over its 512 rows of Wo.
Host sums the two partials per batch and adds bo.

Mask insight: the mask is a padding mask over key positions (mask=1 adds
-1e9 to the whole logit column -> exp underflows to exactly 0 in fp32,
identically to the reference).  Masked K/V rows therefore contribute
nothing; the host gathers only unmasked rows, padding to a multiple of
256 with rows whose logits are forced to -1e9 via a per-partition bias.
This halves attention FLOPs and exp work on average.

All matmuls run in float32r (fp32 stored/streamed in a PE-friendly form,
full bf16-rate at moving-dim>=256, ~1.5e-4 matmul error vs 2.4e-3 bf16).
"""

import numpy as np

import concourse.bass as bass  # noqa: F401  (bass registers engine types)
import concourse.mybir as mybir
from concourse import bacc
from concourse.tile import TileContext
from concourse.bass_utils import run_bass_kernel_spmd

D = 1024
H = 16
DH = 64
B = 4
S = 2048
NCORE = 8
HPC = 8            # heads per core
COLS = HPC * DH    # 512 projection columns per core
SCALE = 1.0 / np.sqrt(DH)

f32 = mybir.dt.float32
f32r = mybir.dt.float32r
Exp = mybir.ActivationFunctionType.Exp
Ident = mybir.ActivationFunctionType.Identity

_prog_cache = {}


def _k_chunks(n):
    """Split n (multiple of 256) into chunks <=512, each >=256."""
    return [512] * (n // 512) + ([256] if n % 512 else [])


def _build(skp):
    nkt = skp // 128
    kch = _k_chunks(skp)

    nc = bacc.Bacc("TRN2", target_bir_lowering=False, debug=False,
                   num_devices=NCORE)

    xqT = nc.dram_tensor("xqT", [D, S], f32r, kind="ExternalInput")
    xkT = nc.dram_tensor("xkT", [D, skp], f32r, kind="ExternalInput")
    xvT = nc.dram_tensor("xvT", [D, skp], f32r, kind="ExternalInput")
    wq = nc.dram_tensor("wq", [D, COLS], f32r, kind="ExternalInput")
    wk = nc.dram_tensor("wk", [D, COLS], f32r, kind="ExternalInput")
    wvx = nc.dram_tensor("wvx", [D, HPC * 65], f32r, kind="ExternalInput")
    wo = nc.dram_tensor("wo", [COLS, D], f32r, kind="ExternalInput")
    bqk = nc.dram_tensor("bqk", [128, 8], f32, kind="ExternalInput")
    bvx = nc.dram_tensor("bvx", [HPC * 65], f32r, kind="ExternalInput")
    mbias = nc.dram_tensor("mbias", [128, skp // 128], f32,
                           kind="ExternalInput")
    onesd = nc.dram_tensor("onesd", [128, 128], f32r, kind="ExternalInput")
    part = nc.dram_tensor("part", [S, D], f32, kind="ExternalOutput")

    with TileContext(nc) as tc, \
         nc.allow_low_precision(reason="float32r matmul pipeline by design"):
        with tc.tile_pool(name="resid", bufs=1) as rp:
            # long-lived SBUF residents
            qT = [rp.tile([128, S], f32r, tag=f"qT{p}", name=f"qT{p}")
                  for p in range(4)]
            kT = [rp.tile([128, skp], f32r, tag=f"kT{p}", name=f"kT{p}")
                  for p in range(4)]
            vE = [[rp.tile([128, 4 * 65], f32r, tag=f"vE{c}_{t}",
                           name=f"vE{c}_{t}")
                   for t in range(nkt)] for c in range(2)]
            OT = [rp.tile([128, S], f32r, tag=f"OT{p}", name=f"OT{p}")
                  for p in range(4)]
            ones = rp.tile([128, 128], f32r, tag="ones")
            nc.sync.dma_start(out=ones[:], in_=onesd[:])
            # packed per-partition constants: one DMA each (mbias/bq/bk are
            # shipped host-side as [128, n] column packs)
            mbtile = rp.tile([128, nkt], f32, tag="mbtile")
            nc.sync.dma_start(out=mbtile[:], in_=mbias[:])
            mbt = [mbtile[:, t:t + 1] for t in range(nkt)]
            qkb = rp.tile([128, 8], f32, tag="qkb")
            nc.sync.dma_start(out=qkb[:], in_=bqk[:])
            qbt = [qkb[:, m:m + 1] for m in range(4)]
            kbt = [qkb[:, 4 + m:5 + m] for m in range(4)]
            bvt = rp.tile([128, HPC * 65], f32r, tag="bvt")
            nc.sync.dma_start(out=bvt[0:1, :], in_=bvx[None, :])
            # output-projection weights, resident (prefetch early)
            wot = [[None] * 2 for _ in range(4)]
            for k in range(4):
                for n in range(2):
                    w = rp.tile([128, 512], f32r, tag=f"wo{k}{n}",
                                name=f"wo{k}{n}")
                    nc.scalar.dma_start(
                        out=w[:], in_=wo[k * 128:(k + 1) * 128,
                                         n * 512:(n + 1) * 512])
                    wot[k][n] = w

            # ---- Phase A/B: q^T and k^T projections (weights stationary) --
            for (wdr, xdr, dst, bias_t, chunks) in (
                    (wq, xqT, qT, qbt, [512] * 4),
                    (wk, xkT, kT, kbt, kch)):
                with (tc.tile_pool(name="wst", bufs=34) as wp,
                      tc.tile_pool(name="xmov", bufs=3) as xp,
                      tc.tile_pool(name="psA", bufs=2, space="PSUM") as pp):
                    wt = [[None] * 8 for _ in range(4)]
                    for m in range(4):
                        for k in range(8):
                            w = wp.tile([128, 128], f32r, tag="w")
                            nc.scalar.dma_start(
                                out=w[:],
                                in_=wdr[k * 128:(k + 1) * 128, m * 128:(m + 1) * 128])
                            wt[m][k] = w
                    off = 0
                    for csz in chunks:
                        xts = []
                        for k in range(8):
                            xt = xp.tile([128, 512], f32r, tag="x")
                            nc.sync.dma_start(out=xt[:, :csz],
                                              in_=xdr[k * 128:(k + 1) * 128,
                                                      off:off + csz])
                            xts.append(xt)
                        for m in range(4):
                            ps = pp.tile([128, 512], f32, tag=f"ps{m}")
                            for k in range(8):
                                nc.tensor.matmul(ps[:, :csz], wt[m][k][:],
                                                 xts[k][:, :csz],
                                                 start=(k == 0), stop=(k == 7))
                            nc.scalar.activation(dst[m][:, off:off + csz],
                                                 ps[:, :csz], Ident,
                                                 bias=bias_t[m])
                        off += csz

            # ---- Phase C: v projection, natural layout + ones columns ----
            # wvx columns: per head h: 64 v-columns then one zero column;
            # bvx has 1.0 at the ones slots -> psum = v | sums-ones directly.
            with (tc.tile_pool(name="wvp", bufs=8) as wvp,
                  tc.tile_pool(name="xvst", bufs=16) as xvp,
                  tc.tile_pool(name="psC", bufs=2, space="PSUM") as pp):
                wvt = []
                for k in range(8):
                    w = wvp.tile([128, HPC * 65], f32r, tag="wv")
                    nc.scalar.dma_start(out=w[:], in_=wvx[k * 128:(k + 1) * 128, :])
                    wvt.append(w)
                xv4 = {}
                for t in range(nkt):
                    g, gi = divmod(t, 4)
                    if gi == 0:
                        gw = min(512, skp - g * 512)
                        xv4 = {}
                        for k in range(8):
                            xt = xvp.tile([128, 512], f32r, tag="xv",
                                          name=f"xv{g}_{k}")
                            nc.gpsimd.dma_start(out=xt[:, :gw],
                                                in_=xvT[k * 128:(k + 1) * 128,
                                                        g * 512:g * 512 + gw])
                            xv4[k] = xt
                    xvs = [xv4[k][:, gi * 128:(gi + 1) * 128] for k in range(8)]
                    for c in range(2):
                        ps = pp.tile([128, 260], f32, tag=f"c{c}")
                        sl = slice(c * 260, (c + 1) * 260)
                        for k in range(8):
                            nc.tensor.matmul(ps[:], xvs[k], wvt[k][:, sl],
                                             start=(k == 0), stop=False)
                        # bias + ones row via K=1 matmul with a ones row
                        nc.tensor.matmul(ps[:], ones[0:1, :], bvt[0:1, sl],
                                         start=False, stop=True)
                        nc.scalar.activation(vE[c][t][:], ps[:], Ident)

            # ---- Phase D+E: attention (pipelined) + interleaved out-proj
            # Per (pair, qc512): logits for heads 2p/2p+1 land in one
            # [128,1024] PSUM tile (row-packed K=64 matmuls), ONE exp
            # covers both heads, PV accumulates [65,512] per head with the
            # ones-column providing softmax denominators.  The t-loop is
            # software-pipelined (logits t+2 emitted between PV steps), and
            # each pair's normalize is deferred past the next pair's first
            # logits so the PE never head-of-line blocks on the DVE chain.
            # After all 4 pairs of a q-chunk, the output projection for its
            # 4 S-tiles runs as a dense PE burst (keeps HAM warm, streams
            # results out early).
            with (tc.tile_pool(name="pT", bufs=3) as ptp,
                  tc.tile_pool(name="nrm", bufs=4) as nrp,
                  tc.tile_pool(name="oevac", bufs=3) as oev,
                  tc.tile_pool(name="psL", bufs=2, space="PSUM") as psL,
                  tc.tile_pool(name="psO", bufs=2, space="PSUM") as psO):

                def make_lgt(p, qs):
                    def lgt(t):
                        ks = slice(t * 128, (t + 1) * 128)
                        L = psL.tile([128, 1024], f32, tag="ps",
                                     name=f"L{t}")
                        nc.tensor.matmul(L[:, 0:512], kT[p][0:64, ks],
                                         qT[p][0:64, qs], start=True,
                                         stop=True, tile_position=(0, 0))
                        nc.tensor.matmul(L[:, 512:1024], kT[p][64:128, ks],
                                         qT[p][64:128, qs], start=True,
                                         stop=True, tile_position=(64, 0))
                        return L
                    return lgt

                def make_norm(p, qs, oA, oB):
                    def norm():
                        for half, o in ((0, oA), (1, oB)):
                            sm = nrp.tile([128, 512], f32r, tag="sm",
                                          name=f"sm{half}")
                            nc.vector.tensor_copy(sm[64:65, :], o[64:65, :])
                            bc = psL.tile([64, 512], f32, tag="ps",
                                          name=f"bc{half}")
                            nc.tensor.matmul(bc[:], ones[64:65, 0:64],
                                             sm[64:65, :], start=True,
                                             stop=True)
                            rec = nrp.tile([64, 512], f32, tag="rec",
                                           name=f"rec{half}")
                            nc.vector.reciprocal_approx_fast(out=rec[:],
                                                             in_=bc[:])
                            nc.vector.tensor_tensor(
                                out=OT[p][half * 64:(half + 1) * 64, qs],
                                in0=o[0:64, :], in1=rec[:],
                                op=mybir.AluOpType.mult)
                    return norm

                pending_norm = None
                for qc in range(4):
                    q0 = qc * 512
                    qs = slice(q0, q0 + 512)
                    for p in range(4):
                        hA, hB = 2 * p, 2 * p + 1
                        vA = (vE[hA // 4], (hA % 4) * 65)
                        vB = (vE[hB // 4], (hB % 4) * 65)
                        lgt = make_lgt(p, qs)
                        oA = psO.tile([65, 512], f32, tag="oA")
                        oB = psO.tile([65, 512], f32, tag="oB")
                        Ls = {0: lgt(0)}
                        if nkt > 1:
                            Ls[1] = lgt(1)
                        if pending_norm is not None:
                            pending_norm()
                        for t in range(nkt):
                            pTt = ptp.tile([128, 1024], f32r, tag="pT")
                            nc.scalar.activation(pTt[:], Ls.pop(t)[:], Exp,
                                                 bias=mbt[t], scale=SCALE)
                            nc.tensor.matmul(oA[:],
                                             vA[0][t][:, vA[1]:vA[1] + 65],
                                             pTt[:, 0:512], start=(t == 0),
                                             stop=(t == nkt - 1))
                            nc.tensor.matmul(oB[:],
                                             vB[0][t][:, vB[1]:vB[1] + 65],
                                             pTt[:, 512:1024], start=(t == 0),
                                             stop=(t == nkt - 1))
                            if t + 2 < nkt:
                                Ls[t + 2] = lgt(t + 2)
                        pending_norm = make_norm(p, qs, oA, oB)
                    # flush the last pair's normalize, then project this
                    # q-chunk's 4 S-tiles (dense PE burst)
                    pending_norm()
                    pending_norm = None
                    for st in range(4 * qc, 4 * qc + 4):
                        ss = slice(st * 128, (st + 1) * 128)
                        po = psL.tile([128, 1024], f32, tag="ps",
                                      name=f"po{st}")
                        for k in range(4):
                            for n in range(2):
                                nc.tensor.matmul(po[:, n * 512:(n + 1) * 512],
                                                 OT[k][:, ss], wot[k][n][:],
                                                 start=(k == 0), stop=(k == 3))
                        ot = oev.tile([128, 1024], f32, tag="oe")
                        nc.vector.tensor_copy(ot[:], po[:])
                        nc.sync.dma_start(out=part[ss, :], in_=ot[:])

    nc.compile()
    return nc


def prepare(query, key, value, mask, Wq, bq, Wk, bk, Wv, bv, Wo, bo):
    """Build/fetch the compiled program and the per-core input maps."""
    query = np.asarray(query, np.float32)
    key = np.asarray(key, np.float32)
    value = np.asarray(value, np.float32)
    mask = np.asarray(mask, np.float32)
    Wq, bq = np.asarray(Wq, np.float32), np.asarray(bq, np.float32)
    Wk, bk = np.asarray(Wk, np.float32), np.asarray(bk, np.float32)
    Wv, bv = np.asarray(Wv, np.float32), np.asarray(bv, np.float32)
    Wo, bo = np.asarray(Wo, np.float32), np.asarray(bo, np.float32)

    idx = [np.nonzero(mask[b, 0, 0] == 0.0)[0] for b in range(B)]
    sk_max = max(1, max(len(i) for i in idx))
    skp = max(256, int(-(-sk_max // 256) * 256))

    if skp not in _prog_cache:
        _prog_cache[skp] = _build(skp)
    nc = _prog_cache[skp]

    onesv = np.ones((128, 128), np.float32)
    in_maps = []
    for c in range(NCORE):
        b, hg = divmod(c, 2)
        cols = slice(hg * COLS, (hg + 1) * COLS)
        sk = len(idx[b])
        xk = np.zeros((skp, D), np.float32)
        xk[:sk] = key[b][idx[b]]
        xv = np.zeros((skp, D), np.float32)
        xv[:sk] = value[b][idx[b]]
        wv_s = Wv[:, cols]
        bv_s = bv[cols]
        wvx = np.zeros((D, HPC * 65), np.float32)
        bvx = np.zeros(HPC * 65, np.float32)
        for h in range(HPC):
            wvx[:, h * 65:h * 65 + 64] = wv_s[:, h * 64:(h + 1) * 64]
            bvx[h * 65:h * 65 + 64] = bv_s[h * 64:(h + 1) * 64]
            bvx[h * 65 + 64] = 1.0
        mb = np.full(skp, -1e9, np.float32)
        mb[:sk] = 0.0
        bqk = np.empty((128, 8), np.float32)
        for m in range(4):
            bqk[:, m] = bq[cols][m * 128:(m + 1) * 128]
            bqk[:, 4 + m] = bk[cols][m * 128:(m + 1) * 128]
        in_maps.append({
            "xqT": np.ascontiguousarray(query[b].T),
            "xkT": np.ascontiguousarray(xk.T),
            "xvT": np.ascontiguousarray(xv.T),
            "wq": np.ascontiguousarray(Wq[:, cols]),
            "wk": np.ascontiguousarray(Wk[:, cols]),
            "wvx": wvx,
            "wo": np.ascontiguousarray(Wo[cols, :]),
            "bqk": bqk,
            "bvx": bvx,
            "mbias": np.ascontiguousarray(mb.reshape(-1, 128).T),
            "onesd": onesv,
        })

    return nc, in_maps


def kernel(query, key, value, mask, Wq, bq, Wk, bk, Wv, bv, Wo, bo):
    nc, in_maps = prepare(query, key, value, mask, Wq, bq, Wk, bk,
                          Wv, bv, Wo, bo)
    res = run_bass_kernel_spmd(nc, in_maps, list(range(NCORE)))
    bo = np.asarray(bo, np.float32)
    out = np.empty((B, S, D), np.float32)
    for b in range(B):
        out[b] = res.results[2 * b]["part"] + res.results[2 * b + 1]["part"] + bo
    return out


# revision 18
# speedup vs baseline: 1.6623x; 1.0468x over previous
"""MultiHeadAttention Trainium2 kernel (8 NeuronCores, SPMD).

Sharding: core c -> (batch b = c//2, head-group hg = c%2 of 8 heads).
Each core computes q/k/v projections for its 512 head-columns, attention
for its 8 heads, and a partial output projection over its 512 rows of Wo.
Host sums the two partials per batch and adds bo.

Mask insight: the mask is a padding mask over key positions (mask=1 adds
-1e9 to the whole logit column -> exp underflows to exactly 0 in fp32,
identically to the reference).  Masked K/V rows therefore contribute
nothing; the host gathers only unmasked rows, padding to a multiple of
256 with rows whose logits are forced to -1e9 via a per-partition bias.
This halves attention FLOPs and exp work on average.

All matmuls run in float32r (fp32 stored/streamed in a PE-friendly form,
full bf16-rate at moving-dim>=256, ~1.5e-4 matmul error vs 2.4e-3 bf16).
"""

import numpy as np

import concourse.bass as bass  # noqa: F401  (bass registers engine types)
import concourse.mybir as mybir
from concourse import bacc
from concourse.tile import TileContext
from concourse.bass_utils import run_bass_kernel_spmd

D = 1024
H = 16
DH = 64
B = 4
S = 2048
NCORE = 8
HPC = 8            # heads per core
COLS = HPC * DH    # 512 projection columns per core
SCALE = 1.0 / np.sqrt(DH)

f32 = mybir.dt.float32
f32r = mybir.dt.float32r
Exp = mybir.ActivationFunctionType.Exp
Ident = mybir.ActivationFunctionType.Identity

_prog_cache = {}


def _k_chunks(n):
    """Split n (multiple of 256) into chunks <=512, each >=256."""
    return [512] * (n // 512) + ([256] if n % 512 else [])


def _build(skp):
    nkt = skp // 128
    kch = _k_chunks(skp)

    nc = bacc.Bacc("TRN2", target_bir_lowering=False, debug=False,
                   num_devices=NCORE)

    xqT = nc.dram_tensor("xqT", [D, S], f32r, kind="ExternalInput")
    xkT = nc.dram_tensor("xkT", [D, skp], f32r, kind="ExternalInput")
    xvT = nc.dram_tensor("xvT", [D, skp], f32r, kind="ExternalInput")
    wq = nc.dram_tensor("wq", [D, COLS], f32r, kind="ExternalInput")
    wk = nc.dram_tensor("wk", [D, COLS], f32r, kind="ExternalInput")
    wvx = nc.dram_tensor("wvx", [D, HPC * 65], f32r, kind="ExternalInput")
    wo = nc.dram_tensor("wo", [COLS, D], f32r, kind="ExternalInput")
    bqk = nc.dram_tensor("bqk", [128, 8], f32, kind="ExternalInput")
    bvx = nc.dram_tensor("bvx", [HPC * 65], f32r, kind="ExternalInput")
    mbias = nc.dram_tensor("mbias", [128, skp // 128], f32,
                           kind="ExternalInput")
    onesd = nc.dram_tensor("onesd", [128, 128], f32r, kind="ExternalInput")
    part = nc.dram_tensor("part", [S, D], f32, kind="ExternalOutput")

    with TileContext(nc) as tc, \
         nc.allow_low_precision(reason="float32r matmul pipeline by design"):
        with tc.tile_pool(name="resid", bufs=1) as rp:
            # long-lived SBUF residents
            qT = [rp.tile([128, S], f32r, tag=f"qT{p}", name=f"qT{p}")
                  for p in range(4)]
            kT = [rp.tile([128, skp], f32r, tag=f"kT{p}", name=f"kT{p}")
                  for p in range(4)]
            vE = [[rp.tile([128, 4 * 65], f32r, tag=f"vE{c}_{t}",
                           name=f"vE{c}_{t}")
                   for t in range(nkt)] for c in range(2)]
            OT = [rp.tile([128, S], f32r, tag=f"OT{p}", name=f"OT{p}")
                  for p in range(4)]
            ones = rp.tile([128, 128], f32r, tag="ones")
            nc.sync.dma_start(out=ones[:], in_=onesd[:])
            # HAM warmup: ~4us of dense junk matmuls so the PE clock is at
            # 2.4GHz (K=8/8) by the time real work starts; runs during the
            # initial input DMAs which would otherwise leave the PE idle.
            with tc.tile_pool(name="wup", bufs=1, space="PSUM") as wup:
                wu = wup.tile([128, 128], f32, tag="wu")
                ob = ones[:].bitcast(mybir.dt.bfloat16)
                for _ in range(40):
                    nc.tensor.matmul(wu[:], ob[:, 0:128], ob[:, 0:128],
                                     start=True, stop=True)
            # packed per-partition constants: one DMA each (mbias/bq/bk are
            # shipped host-side as [128, n] column packs)
            mbtile = rp.tile([128, nkt], f32, tag="mbtile")
            nc.sync.dma_start(out=mbtile[:], in_=mbias[:])
            mbt = [mbtile[:, t:t + 1] for t in range(nkt)]
            qkb = rp.tile([128, 8], f32, tag="qkb")
            nc.sync.dma_start(out=qkb[:], in_=bqk[:])
            qbt = [qkb[:, m:m + 1] for m in range(4)]
            kbt = [qkb[:, 4 + m:5 + m] for m in range(4)]
            bvt = rp.tile([128, HPC * 65], f32r, tag="bvt")
            nc.sync.dma_start(out=bvt[0:1, :], in_=bvx[None, :])
            # output-projection weights, resident (prefetch early)
            wot = [[None] * 2 for _ in range(4)]
            for k in range(4):
                for n in range(2):
                    w = rp.tile([128, 512], f32r, tag=f"wo{k}{n}",
                                name=f"wo{k}{n}")
                    nc.scalar.dma_start(
                        out=w[:], in_=wo[k * 128:(k + 1) * 128,
                                         n * 512:(n + 1) * 512])
                    wot[k][n] = w

            # ---- Phase A/B: q^T and k^T projections (weights stationary) --
            for (wdr, xdr, dst, bias_t, chunks) in (
                    (wq, xqT, qT, qbt, [512] * 4),
                    (wk, xkT, kT, kbt, kch)):
                with (tc.tile_pool(name="wst", bufs=34) as wp,
                      tc.tile_pool(name="xmov", bufs=6) as xp,
                      tc.tile_pool(name="psA", bufs=2, space="PSUM") as pp):
                    wt = [[None] * 8 for _ in range(4)]
                    for m in range(4):
                        for k in range(8):
                            w = wp.tile([128, 128], f32r, tag="w")
                            nc.scalar.dma_start(
                                out=w[:],
                                in_=wdr[k * 128:(k + 1) * 128, m * 128:(m + 1) * 128])
                            wt[m][k] = w
                    off = 0
                    for csz in chunks:
                        xts = []
                        for k in range(8):
                            xt = xp.tile([128, 512], f32r, tag="x")
                            eng = nc.sync if k % 2 == 0 else nc.scalar
                            eng.dma_start(out=xt[:, :csz],
                                          in_=xdr[k * 128:(k + 1) * 128,
                                                  off:off + csz])
                            xts.append(xt)
                        for m in range(4):
                            ps = pp.tile([128, 512], f32, tag=f"ps{m}")
                            for k in range(8):
                                nc.tensor.matmul(ps[:, :csz], wt[m][k][:],
                                                 xts[k][:, :csz],
                                                 start=(k == 0), stop=(k == 7))
                            nc.scalar.activation(dst[m][:, off:off + csz],
                                                 ps[:, :csz], Ident,
                                                 bias=bias_t[m])
                        off += csz

            # ---- Phase C: v projection, natural layout + ones columns ----
            # wvx columns: per head h: 64 v-columns then one zero column;
            # bvx has 1.0 at the ones slots -> psum = v | sums-ones directly.
            with (tc.tile_pool(name="wvp", bufs=8) as wvp,
                  tc.tile_pool(name="xvst", bufs=16) as xvp,
                  tc.tile_pool(name="psC", bufs=2, space="PSUM") as pp):
                wvt = []
                for k in range(8):
                    w = wvp.tile([128, HPC * 65], f32r, tag="wv")
                    nc.scalar.dma_start(out=w[:], in_=wvx[k * 128:(k + 1) * 128, :])
                    wvt.append(w)
                xv4 = {}
                for t in range(nkt):
                    g, gi = divmod(t, 4)
                    if gi == 0:
                        gw = min(512, skp - g * 512)
                        xv4 = {}
                        for k in range(8):
                            xt = xvp.tile([128, 512], f32r, tag="xv",
                                          name=f"xv{g}_{k}")
                            nc.gpsimd.dma_start(out=xt[:, :gw],
                                                in_=xvT[k * 128:(k + 1) * 128,
                                                        g * 512:g * 512 + gw])
                            xv4[k] = xt
                    xvs = [xv4[k][:, gi * 128:(gi + 1) * 128] for k in range(8)]
                    for c in range(2):
                        ps = pp.tile([128, 260], f32, tag=f"c{c}")
                        sl = slice(c * 260, (c + 1) * 260)
                        for k in range(8):
                            nc.tensor.matmul(ps[:], xvs[k], wvt[k][:, sl],
                                             start=(k == 0), stop=False)
                        # bias + ones row via K=1 matmul with a ones row
                        nc.tensor.matmul(ps[:], ones[0:1, :], bvt[0:1, sl],
                                         start=False, stop=True)
                        nc.scalar.activation(vE[c][t][:], ps[:], Ident)

            # ---- Phase D+E: attention (pipelined) + interleaved out-proj
            # Per (pair, qc512): logits for heads 2p/2p+1 land in one
            # [128,1024] PSUM tile (row-packed K=64 matmuls), ONE exp
            # covers both heads, PV accumulates [65,512] per head with the
            # ones-column providing softmax denominators.  The t-loop is
            # software-pipelined (logits t+2 emitted between PV steps), and
            # each pair's normalize is deferred past the next pair's first
            # logits so the PE never head-of-line blocks on the DVE chain.
            # After all 4 pairs of a q-chunk, the output projection for its
            # 4 S-tiles runs as a dense PE burst (keeps HAM warm, streams
            # results out early).
            with (tc.tile_pool(name="pT", bufs=3) as ptp,
                  tc.tile_pool(name="nrm", bufs=4) as nrp,
                  tc.tile_pool(name="oevac", bufs=3) as oev,
                  tc.tile_pool(name="psL", bufs=2, space="PSUM") as psL,
                  tc.tile_pool(name="psO", bufs=2, space="PSUM") as psO):

                def make_lgt(p, qs):
                    def lgt(t):
                        ks = slice(t * 128, (t + 1) * 128)
                        L = psL.tile([128, 1024], f32, tag="ps",
                                     name=f"L{t}")
                        nc.tensor.matmul(L[:, 0:512], kT[p][0:64, ks],
                                         qT[p][0:64, qs], start=True,
                                         stop=True, tile_position=(0, 0))
                        nc.tensor.matmul(L[:, 512:1024], kT[p][64:128, ks],
                                         qT[p][64:128, qs], start=True,
                                         stop=True, tile_position=(64, 0))
                        return L
                    return lgt

                def make_norm(p, qs, oA, oB):
                    def norm():
                        for half, o in ((0, oA), (1, oB)):
                            sm = nrp.tile([128, 512], f32r, tag="sm",
                                          name=f"sm{half}")
                            nc.vector.tensor_copy(sm[64:65, :], o[64:65, :])
                            bc = psL.tile([64, 512], f32, tag="ps",
                                          name=f"bc{half}")
                            nc.tensor.matmul(bc[:], ones[64:65, 0:64],
                                             sm[64:65, :], start=True,
                                             stop=True)
                            rec = nrp.tile([64, 512], f32, tag="rec",
                                           name=f"rec{half}")
                            nc.vector.reciprocal_approx_fast(out=rec[:],
                                                             in_=bc[:])
                            nc.vector.tensor_tensor(
                                out=OT[p][half * 64:(half + 1) * 64, qs],
                                in0=o[0:64, :], in1=rec[:],
                                op=mybir.AluOpType.mult)
                    return norm

                pending_norm = None
                for qc in range(4):
                    q0 = qc * 512
                    qs = slice(q0, q0 + 512)
                    for p in range(4):
                        hA, hB = 2 * p, 2 * p + 1
                        vA = (vE[hA // 4], (hA % 4) * 65)
                        vB = (vE[hB // 4], (hB % 4) * 65)
                        lgt = make_lgt(p, qs)
                        oA = psO.tile([65, 512], f32, tag="oA")
                        oB = psO.tile([65, 512], f32, tag="oB")
                        Ls = {0: lgt(0)}
                        if nkt > 1:
                            Ls[1] = lgt(1)
                        if pending_norm is not None:
                            pending_norm()
                        for t in range(nkt):
                            pTt = ptp.tile([128, 1024], f32r, tag="pT")
                            nc.scalar.activation(pTt[:], Ls.pop(t)[:], Exp,
                                                 bias=mbt[t], scale=SCALE)
                            nc.tensor.matmul(oA[:],
                                             vA[0][t][:, vA[1]:vA[1] + 65],
                                             pTt[:, 0:512], start=(t == 0),
                                             stop=(t == nkt - 1))
                            nc.tensor.matmul(oB[:],
                                             vB[0][t][:, vB[1]:vB[1] + 65],
                                             pTt[:, 512:1024], start=(t == 0),
                                             stop=(t == nkt - 1))
                            if t + 2 < nkt:
                                Ls[t + 2] = lgt(t + 2)
                        pending_norm = make_norm(p, qs, oA, oB)
                    # flush the last pair's normalize, then project this
                    # q-chunk's 4 S-tiles (dense PE burst)
                    pending_norm()
                    pending_norm = None
                    for st in range(4 * qc, 4 * qc + 4):
                        ss = slice(st * 128, (st + 1) * 128)
                        po = psL.tile([128, 1024], f32, tag="ps",
                                      name=f"po{st}")
                        for k in range(4):
                            for n in range(2):
                                nc.tensor.matmul(po[:, n * 512:(n + 1) * 512],
                                                 OT[k][:, ss], wot[k][n][:],
                                                 start=(k == 0), stop=(k == 3))
                        ot = oev.tile([128, 1024], f32, tag="oe")
                        nc.vector.tensor_copy(ot[:], po[:])
                        nc.sync.dma_start(out=part[ss, :], in_=ot[:])

    nc.compile()
    return nc


def prepare(query, key, value, mask, Wq, bq, Wk, bk, Wv, bv, Wo, bo):
    """Build/fetch the compiled program and the per-core input maps."""
    query = np.asarray(query, np.float32)
    key = np.asarray(key, np.float32)
    value = np.asarray(value, np.float32)
    mask = np.asarray(mask, np.float32)
    Wq, bq = np.asarray(Wq, np.float32), np.asarray(bq, np.float32)
    Wk, bk = np.asarray(Wk, np.float32), np.asarray(bk, np.float32)
    Wv, bv = np.asarray(Wv, np.float32), np.asarray(bv, np.float32)
    Wo, bo = np.asarray(Wo, np.float32), np.asarray(bo, np.float32)

    idx = [np.nonzero(mask[b, 0, 0] == 0.0)[0] for b in range(B)]
    sk_max = max(1, max(len(i) for i in idx))
    skp = max(256, int(-(-sk_max // 256) * 256))

    if skp not in _prog_cache:
        _prog_cache[skp] = _build(skp)
    nc = _prog_cache[skp]

    onesv = np.ones((128, 128), np.float32)
    in_maps = []
    for c in range(NCORE):
        b, hg = divmod(c, 2)
        cols = slice(hg * COLS, (hg + 1) * COLS)
        sk = len(idx[b])
        xk = np.zeros((skp, D), np.float32)
        xk[:sk] = key[b][idx[b]]
        xv = np.zeros((skp, D), np.float32)
        xv[:sk] = value[b][idx[b]]
        wv_s = Wv[:, cols]
        bv_s = bv[cols]
        wvx = np.zeros((D, HPC * 65), np.float32)
        bvx = np.zeros(HPC * 65, np.float32)
        for h in range(HPC):
            wvx[:, h * 65:h * 65 + 64] = wv_s[:, h * 64:(h + 1) * 64]
            bvx[h * 65:h * 65 + 64] = bv_s[h * 64:(h + 1) * 64]
            bvx[h * 65 + 64] = 1.0
        mb = np.full(skp, -1e9, np.float32)
        mb[:sk] = 0.0
        bqk = np.empty((128, 8), np.float32)
        for m in range(4):
            bqk[:, m] = bq[cols][m * 128:(m + 1) * 128]
            bqk[:, 4 + m] = bk[cols][m * 128:(m + 1) * 128]
        in_maps.append({
            "xqT": np.ascontiguousarray(query[b].T),
            "xkT": np.ascontiguousarray(xk.T),
            "xvT": np.ascontiguousarray(xv.T),
            "wq": np.ascontiguousarray(Wq[:, cols]),
            "wk": np.ascontiguousarray(Wk[:, cols]),
            "wvx": wvx,
            "wo": np.ascontiguousarray(Wo[cols, :]),
            "bqk": bqk,
            "bvx": bvx,
            "mbias": np.ascontiguousarray(mb.reshape(-1, 128).T),
            "onesd": onesv,
        })

    return nc, in_maps


def kernel(query, key, value, mask, Wq, bq, Wk, bk, Wv, bv, Wo, bo):
    nc, in_maps = prepare(query, key, value, mask, Wq, bq, Wk, bk,
                          Wv, bv, Wo, bo)
    res = run_bass_kernel_spmd(nc, in_maps, list(range(NCORE)))
    bo = np.asarray(bo, np.float32)
    out = np.empty((B, S, D), np.float32)
    for b in range(B):
        out[b] = res.results[2 * b]["part"] + res.results[2 * b + 1]["part"] + bo
    return out


# revision 19
# speedup vs baseline: 1.6739x; 1.0070x over previous
"""MultiHeadAttention Trainium2 kernel (8 NeuronCores, SPMD).

Sharding: core c -> (batch b = c//2, head-group hg = c%2 of 8 heads).
Each core computes q/k/v projections for its 512 head-columns, attention
for its 8 heads, and a partial output projection over its 512 rows of Wo.
Host sums the two partials per batch and adds bo.

Mask insight: the mask is a padding mask over key positions (mask=1 adds
-1e9 to the whole logit column -> exp underflows to exactly 0 in fp32,
identically to the reference).  Masked K/V rows therefore contribute
nothing; the host gathers only unmasked rows, padding to a multiple of
256 with rows whose logits are forced to -1e9 via a per-partition bias.
This halves attention FLOPs and exp work on average.

All matmuls run in float32r (fp32 stored/streamed in a PE-friendly form,
full bf16-rate at moving-dim>=256, ~1.5e-4 matmul error vs 2.4e-3 bf16).
"""

import numpy as np

import concourse.bass as bass  # noqa: F401  (bass registers engine types)
import concourse.mybir as mybir
from concourse import bacc
from concourse.tile import TileContext
from concourse.bass_utils import run_bass_kernel_spmd

D = 1024
H = 16
DH = 64
B = 4
S = 2048
NCORE = 8
HPC = 8            # heads per core
COLS = HPC * DH    # 512 projection columns per core
SCALE = 1.0 / np.sqrt(DH)

f32 = mybir.dt.float32
f32r = mybir.dt.float32r
Exp = mybir.ActivationFunctionType.Exp
Ident = mybir.ActivationFunctionType.Identity

_prog_cache = {}


def _k_chunks(n):
    """Split n (multiple of 256) into chunks <=512, each >=256."""
    return [512] * (n // 512) + ([256] if n % 512 else [])


def _build(skp):
    nkt = skp // 128
    kch = _k_chunks(skp)

    nc = bacc.Bacc("TRN2", target_bir_lowering=False, debug=False,
                   num_devices=NCORE)

    xqT = nc.dram_tensor("xqT", [D, S], f32r, kind="ExternalInput")
    xkT = nc.dram_tensor("xkT", [D, skp], f32r, kind="ExternalInput")
    xvT = nc.dram_tensor("xvT", [D, skp], f32r, kind="ExternalInput")
    wq = nc.dram_tensor("wq", [D, COLS], f32r, kind="ExternalInput")
    wk = nc.dram_tensor("wk", [D, COLS], f32r, kind="ExternalInput")
    wvx = nc.dram_tensor("wvx", [D, HPC * 65], f32r, kind="ExternalInput")
    wo = nc.dram_tensor("wo", [COLS, D], f32r, kind="ExternalInput")
    bqk = nc.dram_tensor("bqk", [128, 8], f32, kind="ExternalInput")
    bvx = nc.dram_tensor("bvx", [HPC * 65], f32r, kind="ExternalInput")
    mbias = nc.dram_tensor("mbias", [128, skp // 128], f32,
                           kind="ExternalInput")
    onesd = nc.dram_tensor("onesd", [128, 128], f32r, kind="ExternalInput")
    part = nc.dram_tensor("part", [S, D], f32, kind="ExternalOutput")

    with TileContext(nc) as tc, \
         nc.allow_low_precision(reason="float32r matmul pipeline by design"):
        with tc.tile_pool(name="resid", bufs=1) as rp:
            # long-lived SBUF residents
            qT = [rp.tile([128, S], f32r, tag=f"qT{p}", name=f"qT{p}")
                  for p in range(4)]
            kT = [rp.tile([128, skp], f32r, tag=f"kT{p}", name=f"kT{p}")
                  for p in range(4)]
            vE = [[rp.tile([128, 4 * 65], f32r, tag=f"vE{c}_{t}",
                           name=f"vE{c}_{t}")
                   for t in range(nkt)] for c in range(2)]
            OT = [rp.tile([128, S], f32r, tag=f"OT{p}", name=f"OT{p}")
                  for p in range(4)]
            ones = rp.tile([128, 128], f32r, tag="ones")
            nc.sync.dma_start(out=ones[:], in_=onesd[:])
            # HAM warmup: ~4us of dense junk matmuls so the PE clock is at
            # 2.4GHz (K=8/8) by the time real work starts; runs during the
            # initial input DMAs which would otherwise leave the PE idle.
            with tc.tile_pool(name="wup", bufs=1, space="PSUM") as wup:
                wu = wup.tile([128, 128], f32, tag="wu")
                ob = ones[:].bitcast(mybir.dt.bfloat16)
                for _ in range(60):
                    nc.tensor.matmul(wu[:], ob[:, 0:128], ob[:, 0:128],
                                     start=True, stop=True)
            # packed per-partition constants: one DMA each (mbias/bq/bk are
            # shipped host-side as [128, n] column packs)
            mbtile = rp.tile([128, nkt], f32, tag="mbtile")
            nc.sync.dma_start(out=mbtile[:], in_=mbias[:])
            mbt = [mbtile[:, t:t + 1] for t in range(nkt)]
            qkb = rp.tile([128, 8], f32, tag="qkb")
            nc.sync.dma_start(out=qkb[:], in_=bqk[:])
            qbt = [qkb[:, m:m + 1] for m in range(4)]
            kbt = [qkb[:, 4 + m:5 + m] for m in range(4)]
            bvt = rp.tile([128, HPC * 65], f32r, tag="bvt")
            nc.sync.dma_start(out=bvt[0:1, :], in_=bvx[None, :])
            # output-projection weights, resident (prefetch early)
            wot = [[None] * 2 for _ in range(4)]
            for k in range(4):
                for n in range(2):
                    w = rp.tile([128, 512], f32r, tag=f"wo{k}{n}",
                                name=f"wo{k}{n}")
                    nc.scalar.dma_start(
                        out=w[:], in_=wo[k * 128:(k + 1) * 128,
                                         n * 512:(n + 1) * 512])
                    wot[k][n] = w

            # ---- Phase A/B: q^T and k^T projections (weights stationary) --
            for (wdr, xdr, dst, bias_t, chunks) in (
                    (wq, xqT, qT, qbt, [512] * 4),
                    (wk, xkT, kT, kbt, kch)):
                with (tc.tile_pool(name="wst", bufs=34) as wp,
                      tc.tile_pool(name="xmov", bufs=16) as xp,
                      tc.tile_pool(name="psA", bufs=2, space="PSUM") as pp):
                    wt = [[None] * 8 for _ in range(4)]
                    for m in range(4):
                        for k in range(8):
                            w = wp.tile([128, 128], f32r, tag="w")
                            nc.scalar.dma_start(
                                out=w[:],
                                in_=wdr[k * 128:(k + 1) * 128, m * 128:(m + 1) * 128])
                            wt[m][k] = w
                    off = 0
                    for csz in chunks:
                        xts = []
                        for k in range(8):
                            xt = xp.tile([128, 512], f32r, tag="x")
                            eng = nc.sync if k % 2 == 0 else nc.scalar
                            eng.dma_start(out=xt[:, :csz],
                                          in_=xdr[k * 128:(k + 1) * 128,
                                                  off:off + csz])
                            xts.append(xt)
                        for m in range(4):
                            ps = pp.tile([128, 512], f32, tag=f"ps{m}")
                            for k in range(8):
                                nc.tensor.matmul(ps[:, :csz], wt[m][k][:],
                                                 xts[k][:, :csz],
                                                 start=(k == 0), stop=(k == 7))
                            nc.scalar.activation(dst[m][:, off:off + csz],
                                                 ps[:, :csz], Ident,
                                                 bias=bias_t[m])
                        off += csz

            # ---- Phase C: v projection, natural layout + ones columns ----
            # wvx columns: per head h: 64 v-columns then one zero column;
            # bvx has 1.0 at the ones slots -> psum = v | sums-ones directly.
            with (tc.tile_pool(name="wvp", bufs=8) as wvp,
                  tc.tile_pool(name="xvst", bufs=16) as xvp,
                  tc.tile_pool(name="psC", bufs=2, space="PSUM") as pp):
                wvt = []
                for k in range(8):
                    w = wvp.tile([128, HPC * 65], f32r, tag="wv")
                    nc.scalar.dma_start(out=w[:], in_=wvx[k * 128:(k + 1) * 128, :])
                    wvt.append(w)
                xv4 = {}
                for t in range(nkt):
                    g, gi = divmod(t, 4)
                    if gi == 0:
                        gw = min(512, skp - g * 512)
                        xv4 = {}
                        for k in range(8):
                            xt = xvp.tile([128, 512], f32r, tag="xv",
                                          name=f"xv{g}_{k}")
                            nc.gpsimd.dma_start(out=xt[:, :gw],
                                                in_=xvT[k * 128:(k + 1) * 128,
                                                        g * 512:g * 512 + gw])
                            xv4[k] = xt
                    xvs = [xv4[k][:, gi * 128:(gi + 1) * 128] for k in range(8)]
                    for c in range(2):
                        ps = pp.tile([128, 260], f32, tag=f"c{c}")
                        sl = slice(c * 260, (c + 1) * 260)
                        for k in range(8):
                            nc.tensor.matmul(ps[:], xvs[k], wvt[k][:, sl],
                                             start=(k == 0), stop=False)
                        # bias + ones row via K=1 matmul with a ones row
                        nc.tensor.matmul(ps[:], ones[0:1, :], bvt[0:1, sl],
                                         start=False, stop=True)
                        nc.scalar.activation(vE[c][t][:], ps[:], Ident)

            # ---- Phase D+E: attention (pipelined) + interleaved out-proj
            # Per (pair, qc512): logits for heads 2p/2p+1 land in one
            # [128,1024] PSUM tile (row-packed K=64 matmuls), ONE exp
            # covers both heads, PV accumulates [65,512] per head with the
            # ones-column providing softmax denominators.  The t-loop is
            # software-pipelined (logits t+2 emitted between PV steps), and
            # each pair's normalize is deferred past the next pair's first
            # logits so the PE never head-of-line blocks on the DVE chain.
            # After all 4 pairs of a q-chunk, the output projection for its
            # 4 S-tiles runs as a dense PE burst (keeps HAM warm, streams
            # results out early).
            with (tc.tile_pool(name="pT", bufs=3) as ptp,
                  tc.tile_pool(name="nrm", bufs=4) as nrp,
                  tc.tile_pool(name="oevac", bufs=3) as oev,
                  tc.tile_pool(name="psL", bufs=3, space="PSUM") as psL,
                  tc.tile_pool(name="psO", bufs=1, space="PSUM") as psO):

                def make_lgt(p, qs):
                    def lgt(t):
                        ks = slice(t * 128, (t + 1) * 128)
                        L = psL.tile([128, 1024], f32, tag="ps",
                                     name=f"L{t}")
                        nc.tensor.matmul(L[:, 0:512], kT[p][0:64, ks],
                                         qT[p][0:64, qs], start=True,
                                         stop=True, tile_position=(0, 0))
                        nc.tensor.matmul(L[:, 512:1024], kT[p][64:128, ks],
                                         qT[p][64:128, qs], start=True,
                                         stop=True, tile_position=(64, 0))
                        return L
                    return lgt

                def make_norm(p, qs, oA, oB):
                    def norm():
                        for half, o in ((0, oA), (1, oB)):
                            sm = nrp.tile([128, 512], f32r, tag="sm",
                                          name=f"sm{half}")
                            nc.vector.tensor_copy(sm[64:65, :], o[64:65, :])
                            bc = psL.tile([64, 512], f32, tag="ps",
                                          name=f"bc{half}")
                            nc.tensor.matmul(bc[:], ones[64:65, 0:64],
                                             sm[64:65, :], start=True,
                                             stop=True)
                            rec = nrp.tile([64, 512], f32, tag="rec",
                                           name=f"rec{half}")
                            nc.vector.reciprocal_approx_fast(out=rec[:],
                                                             in_=bc[:])
                            nc.vector.tensor_tensor(
                                out=OT[p][half * 64:(half + 1) * 64, qs],
                                in0=o[0:64, :], in1=rec[:],
                                op=mybir.AluOpType.mult)
                    return norm

                def make_oproj(qc):
                    tiles = []
                    for st in range(4 * qc, 4 * qc + 4):
                        def op(st=st):
                            ss = slice(st * 128, (st + 1) * 128)
                            po = psL.tile([128, 1024], f32, tag="ps",
                                          name=f"po{st}")
                            for k in range(4):
                                for n in range(2):
                                    nc.tensor.matmul(
                                        po[:, n * 512:(n + 1) * 512],
                                        OT[k][:, ss], wot[k][n][:],
                                        start=(k == 0), stop=(k == 3))
                            ot = oev.tile([128, 1024], f32, tag="oe")
                            nc.vector.tensor_copy(ot[:], po[:])
                            nc.sync.dma_start(out=part[ss, :], in_=ot[:])
                        tiles.append(op)
                    return tiles

                pending_norm = None
                pending_oproj = []
                for qc in range(4):
                    q0 = qc * 512
                    qs = slice(q0, q0 + 512)
                    for p in range(4):
                        hA, hB = 2 * p, 2 * p + 1
                        vA = (vE[hA // 4], (hA % 4) * 65)
                        vB = (vE[hB // 4], (hB % 4) * 65)
                        lgt = make_lgt(p, qs)
                        oA = psO.tile([65, 512], f32, tag="oA")
                        oB = psO.tile([65, 512], f32, tag="oB")
                        Ls = {0: lgt(0)}
                        if nkt > 1:
                            Ls[1] = lgt(1)
                        if pending_norm is not None:
                            pending_norm()
                            pending_norm = None
                        for t in range(nkt):
                            pTt = ptp.tile([128, 1024], f32r, tag="pT")
                            nc.scalar.activation(pTt[:], Ls.pop(t)[:], Exp,
                                                 bias=mbt[t], scale=SCALE)
                            nc.tensor.matmul(oA[:],
                                             vA[0][t][:, vA[1]:vA[1] + 65],
                                             pTt[:, 0:512], start=(t == 0),
                                             stop=(t == nkt - 1))
                            nc.tensor.matmul(oB[:],
                                             vB[0][t][:, vB[1]:vB[1] + 65],
                                             pTt[:, 512:1024], start=(t == 0),
                                             stop=(t == nkt - 1))
                            if t + 2 < nkt:
                                Ls[t + 2] = lgt(t + 2)
                            if pending_oproj and t % 2 == 1:
                                pending_oproj.pop(0)()
                        pending_norm = make_norm(p, qs, oA, oB)
                        if p == 3:
                            while pending_oproj:
                                pending_oproj.pop(0)()
                            pending_oproj = make_oproj(qc)
                # tail: last pair's normalize + last q-chunk's projection
                pending_norm()
                while pending_oproj:
                    pending_oproj.pop(0)()

    nc.compile()
    return nc


def prepare(query, key, value, mask, Wq, bq, Wk, bk, Wv, bv, Wo, bo):
    """Build/fetch the compiled program and the per-core input maps."""
    query = np.asarray(query, np.float32)
    key = np.asarray(key, np.float32)
    value = np.asarray(value, np.float32)
    mask = np.asarray(mask, np.float32)
    Wq, bq = np.asarray(Wq, np.float32), np.asarray(bq, np.float32)
    Wk, bk = np.asarray(Wk, np.float32), np.asarray(bk, np.float32)
    Wv, bv = np.asarray(Wv, np.float32), np.asarray(bv, np.float32)
    Wo, bo = np.asarray(Wo, np.float32), np.asarray(bo, np.float32)

    idx = [np.nonzero(mask[b, 0, 0] == 0.0)[0] for b in range(B)]
    sk_max = max(1, max(len(i) for i in idx))
    skp = max(256, int(-(-sk_max // 256) * 256))

    if skp not in _prog_cache:
        _prog_cache[skp] = _build(skp)
    nc = _prog_cache[skp]

    onesv = np.ones((128, 128), np.float32)
    in_maps = []
    for c in range(NCORE):
        b, hg = divmod(c, 2)
        cols = slice(hg * COLS, (hg + 1) * COLS)
        sk = len(idx[b])
        xk = np.zeros((skp, D), np.float32)
        xk[:sk] = key[b][idx[b]]
        xv = np.zeros((skp, D), np.float32)
        xv[:sk] = value[b][idx[b]]
        wv_s = Wv[:, cols]
        bv_s = bv[cols]
        wvx = np.zeros((D, HPC * 65), np.float32)
        bvx = np.zeros(HPC * 65, np.float32)
        for h in range(HPC):
            wvx[:, h * 65:h * 65 + 64] = wv_s[:, h * 64:(h + 1) * 64]
            bvx[h * 65:h * 65 + 64] = bv_s[h * 64:(h + 1) * 64]
            bvx[h * 65 + 64] = 1.0
        mb = np.full(skp, -1e9, np.float32)
        mb[:sk] = 0.0
        bqk = np.empty((128, 8), np.float32)
        for m in range(4):
            bqk[:, m] = bq[cols][m * 128:(m + 1) * 128]
            bqk[:, 4 + m] = bk[cols][m * 128:(m + 1) * 128]
        in_maps.append({
            "xqT": np.ascontiguousarray(query[b].T),
            "xkT": np.ascontiguousarray(xk.T),
            "xvT": np.ascontiguousarray(xv.T),
            "wq": np.ascontiguousarray(Wq[:, cols]),
            "wk": np.ascontiguousarray(Wk[:, cols]),
            "wvx": wvx,
            "wo": np.ascontiguousarray(Wo[cols, :]),
            "bqk": bqk,
            "bvx": bvx,
            "mbias": np.ascontiguousarray(mb.reshape(-1, 128).T),
            "onesd": onesv,
        })

    return nc, in_maps


def kernel(query, key, value, mask, Wq, bq, Wk, bk, Wv, bv, Wo, bo):
    nc, in_maps = prepare(query, key, value, mask, Wq, bq, Wk, bk,
                          Wv, bv, Wo, bo)
    res = run_bass_kernel_spmd(nc, in_maps, list(range(NCORE)))
    bo = np.asarray(bo, np.float32)
    out = np.empty((B, S, D), np.float32)
    for b in range(B):
        out[b] = res.results[2 * b]["part"] + res.results[2 * b + 1]["part"] + bo
    return out
